# revision 10
# baseline (speedup 1.0000x reference)
"""Trainium2 Bass kernel for nn_LogicLerpInputLayer (segment_reduce).

Computation (reference):
    q = concat([database.sum(axis=1),      # [32, 2048] column sums per relation
                database.sum(axis=2),      # [32, 2048] row sums per relation
                attributes * 2.0])         # [8, 2048]
    quantified = 1 - exp(-q)               # [72, 2048]
    outputs = softmax(weights, axis=0).T @ quantified   # [256, 2048]
    return outputs, quantified

Sharding: database [32, 2048, 2048] is split along the relation axis, 4
relations per core (64 MiB/core — the dominant, memory-bound transfer).
Each core reduces its 4 relations (column sums on the PE via ones-matmul,
row sums on the DVE via free-axis reduce), applies 1-exp(-x), and computes
its partial contribution to the output matmul using its 9 rows of the
softmax numerator (4 colsum rows + 4 rowsum rows + 1 attribute row) with
the shared denominator computed from the full (tiny) weights. The host sums
the 8 partial [256, 2048] outputs and scatters the quantified rows.
"""

import numpy as np

import concourse.bass as bass
import concourse.bacc as bacc
import concourse.mybir as mybir
import concourse.tile as tile
from concourse import masks
from concourse.bass_utils import run_bass_kernel_spmd

N_REL = 32
N_ATTR = 8
WIDTH = 256
N_ENT = 2048
N_CORES = 8
R_PER = N_REL // N_CORES          # 4 relations per core
QROWS = 2 * R_PER + 1             # 9 quantified rows per core
KTOT = 2 * N_REL + N_ATTR         # 72
ROW_TILES = N_ENT // 128          # 16
F32 = mybir.dt.float32

_NC_CACHE = {}

# Test-harness hooks (unused when the kernel is called normally).
TRACE = False
LAST_RESULT = None


def _build_nc(db_bufs: int = 8, reps: int = 1):
    nc = bacc.Bacc("TRN2", target_bir_lowering=False, debug=False)

    db = nc.dram_tensor("db", [R_PER, N_ENT, N_ENT], F32, kind="ExternalInput")
    attr = nc.dram_tensor("attr", [1, N_ENT], F32, kind="ExternalInput")
    wfull = nc.dram_tensor("wfull", [KTOT, WIDTH], F32, kind="ExternalInput")
    wsel = nc.dram_tensor("wsel", [QROWS, WIDTH], F32, kind="ExternalInput")
    q_out = nc.dram_tensor("q_out", [QROWS, N_ENT], F32, kind="ExternalOutput")
    part_out = nc.dram_tensor("part_out", [WIDTH, N_ENT], F32, kind="ExternalOutput")

    AF = mybir.ActivationFunctionType
    ALU = mybir.AluOpType
    AX = mybir.AxisListType

    with tile.TileContext(nc) as tc:
        with (
            tc.tile_pool(name="dbt", bufs=db_bufs) as dbt_pool,
            tc.tile_pool(name="singles", bufs=1) as singles,
            tc.tile_pool(name="rs", bufs=2) as rs_pool,
            tc.tile_pool(name="evac", bufs=2) as evac_pool,
            tc.tile_pool(name="outsb", bufs=4) as out_pool,
            tc.tile_pool(name="cs_ps", bufs=1, space="PSUM") as cs_ps_pool,
            tc.tile_pool(name="tp_ps", bufs=2, space="PSUM") as tp_ps_pool,
            tc.tile_pool(name="mm_ps", bufs=2, space="PSUM") as mm_ps_pool,
        ):
            ones = singles.tile([128, 1], F32)
            nc.gpsimd.memset(ones[:], 1.0)
            ident = singles.tile([128, 128], F32)
            masks.make_identity(nc, ident[:])

            # quantified rows for this core, staged in SBUF for the matmul
            q_sb = singles.tile([QROWS, N_ENT], F32)

            # ---- heavy phase: reduce each relation ----
            for rep, r in [(i, j) for i in range(reps) for j in range(R_PER)]:
                cs_ps = cs_ps_pool.tile([1, N_ENT], F32, tag="cs")
                rs = rs_pool.tile([128, ROW_TILES], F32, tag="rs")
                for t in range(ROW_TILES):
                    dbt = dbt_pool.tile([128, N_ENT], F32, tag="db")
                    nc.sync.dma_start(dbt[:], db[r, 128 * t : 128 * (t + 1), :])
                    # column sums: accumulate ones^T @ tile into PSUM
                    for c in range(4):
                        nc.tensor.matmul(
                            cs_ps[:1, 512 * c : 512 * (c + 1)],
                            ones[:, :1],
                            dbt[:, 512 * c : 512 * (c + 1)],
                            start=(t == 0),
                            stop=(t == ROW_TILES - 1),
                        )
                    # row sums: free-axis reduce on DVE
                    nc.vector.reduce_sum(rs[:, t : t + 1], dbt[:, :], axis=AX.X)

                # colsum row -> q_sb[r] = 1 - exp(-cs)
                e_cs = evac_pool.tile([1, N_ENT], F32, tag="ecs")
                nc.scalar.activation(e_cs[:], cs_ps[:1, :], AF.Exp, scale=-1.0)
                q1 = evac_pool.tile([1, N_ENT], F32, tag="q1")
                nc.vector.tensor_scalar(
                    q1[:], e_cs[:], -1.0, 1.0, op0=ALU.mult, op1=ALU.add
                )
                nc.sync.dma_start(q_sb[r : r + 1, :], q1[:])

                # rowsum row -> q_sb[R_PER + r], via PE transpose + flatten DMA
                e_rs = evac_pool.tile([128, ROW_TILES], F32, tag="ers")
                nc.scalar.activation(e_rs[:], rs[:, :], AF.Exp, scale=-1.0)
                q128 = rs_pool.tile([128, ROW_TILES], F32, tag="q128")
                nc.vector.tensor_scalar(
                    q128[:], e_rs[:], -1.0, 1.0, op0=ALU.mult, op1=ALU.add
                )
                tp = tp_ps_pool.tile([ROW_TILES, 128], F32, tag="tp")
                nc.tensor.transpose(tp[:], q128[:], ident[:])
                flat = evac_pool.tile([ROW_TILES, 128], F32, tag="flat")
                nc.scalar.copy(flat[:], tp[:])
                nc.sync.dma_start(q_sb[R_PER + r : R_PER + r + 1, :], flat[:])

            # ---- attribute row: q_sb[8] = 1 - exp(-2*attr) ----
            a_t = singles.tile([1, N_ENT], F32)
            nc.sync.dma_start(a_t[:], attr[:])
            e_a = evac_pool.tile([1, N_ENT], F32, tag="ea")
            nc.scalar.activation(e_a[:], a_t[:], AF.Exp, scale=-2.0)
            qa = evac_pool.tile([1, N_ENT], F32, tag="qa")
            nc.vector.tensor_scalar(
                qa[:], e_a[:], -1.0, 1.0, op0=ALU.mult, op1=ALU.add
            )
            nc.sync.dma_start(q_sb[QROWS - 1 : QROWS, :], qa[:])

            # quantified rows are an external output as well
            nc.sync.dma_start(q_out[:], q_sb[:])

            # ---- softmax pieces (tiny) ----
            wf = singles.tile([KTOT, WIDTH], F32)
            nc.sync.dma_start(wf[:], wfull[:])
            ef = singles.tile([KTOT, WIDTH], F32)
            nc.scalar.activation(ef[:], wf[:], AF.Exp)
            ws = singles.tile([QROWS, WIDTH], F32)
            nc.sync.dma_start(ws[:], wsel[:])
            es = singles.tile([QROWS, WIDTH], F32)
            nc.scalar.activation(es[:], ws[:], AF.Exp)

            recip = singles.tile([128, 2], F32)
            for h in range(2):
                den_ps = mm_ps_pool.tile([128, 512], F32, tag="mm")
                nc.tensor.matmul(
                    den_ps[:, :1],
                    ef[:, 128 * h : 128 * (h + 1)],
                    ones[:KTOT, :1],
                    start=True,
                    stop=True,
                )
                nc.vector.reciprocal(recip[:, h : h + 1], den_ps[:, :1])

            # ---- partial output matmul: es^T @ q_sb, scaled by recip ----
            for h in range(2):
                for c in range(4):
                    p_ps = mm_ps_pool.tile([128, 512], F32, tag="mm")
                    nc.tensor.matmul(
                        p_ps[:],
                        es[:, 128 * h : 128 * (h + 1)],
                        q_sb[:, 512 * c : 512 * (c + 1)],
                        start=True,
                        stop=True,
                    )
                    ob = out_pool.tile([128, 512], F32, tag="ob")
                    nc.vector.tensor_scalar(
                        ob[:], p_ps[:], recip[:, h : h + 1], None, op0=ALU.mult
                    )
                    nc.sync.dma_start(
                        part_out[128 * h : 128 * (h + 1), 512 * c : 512 * (c + 1)],
                        ob[:],
                    )

    nc.compile()
    return nc


def _get_nc():
    if "nc" not in _NC_CACHE:
        _NC_CACHE["nc"] = _build_nc()
    return _NC_CACHE["nc"]


def _sel_rows(i):
    return (
        list(range(R_PER * i, R_PER * (i + 1)))
        + list(range(N_REL + R_PER * i, N_REL + R_PER * (i + 1)))
        + [2 * N_REL + i]
    )


def kernel(database, attributes, weights, **_):
    database = np.ascontiguousarray(database, dtype=np.float32)
    attributes = np.ascontiguousarray(attributes, dtype=np.float32)
    weights = np.ascontiguousarray(weights, dtype=np.float32)

    nc = _get_nc()
    in_maps = []
    for i in range(N_CORES):
        in_maps.append(
            {
                "db": np.ascontiguousarray(database[R_PER * i : R_PER * (i + 1)]),
                "attr": np.ascontiguousarray(attributes[i : i + 1]),
                "wfull": weights,
                "wsel": np.ascontiguousarray(weights[_sel_rows(i)]),
            }
        )

    res = run_bass_kernel_spmd(
        nc, in_maps, core_ids=list(range(N_CORES)), trace=TRACE
    )
    global LAST_RESULT
    LAST_RESULT = res

    outputs = np.zeros((WIDTH, N_ENT), dtype=np.float32)
    quantified = np.zeros((KTOT, N_ENT), dtype=np.float32)
    for i, r in enumerate(res.results):
        outputs += r["part_out"]
        quantified[_sel_rows(i)] = r["q_out"]
    return outputs, quantified
